# revision 19
# baseline (speedup 1.0000x reference)
"""Trainium2 Bass kernel for nn_MHLMachine (sparse relative-position attention).

Self-contained: kernel(**inputs) takes the FULL inputs (x, generator, Wq, Wv,
Wproj), shards batch-parallel across 8 NeuronCores via bass/PJRT (axon), and
returns the full (8, 1024, 1024) float32 output.

Per-core program (one batch element per core), v2 (transposed-cumsum design):
  phase A: qT = Wq @ x.T (fp32r GEMM -> bf16)
  phase B: per head, w_raw window GEMMs (bf16) -> stg -> DRAM scratch ->
           Toeplitz skew via diagonal-AP DMA -> w2[l', j] bf16 tiles
           (all 4 heads' skew roundtrips overlap phases B+C compute)
  phase C: v = x @ Wv.T (fp32r GEMM, bf16 out, + ones column)
  phase D: per head,
    cumsum^T: psum[j, l]_mjt = w2_m[:,jt]^T @ U  (N=128 matmuls);
      running block-prefix R[j] kept in SBUF (DVE adds psum col 127);
      A'T tile = Relu(psum + R) via ACT bias-add; tril mask on diagonal
      tiles via gpsimd affine_select
    AV: av[l, 0:257] = sum_jt A'T_jt^T @ [v_head | 1]  (rowsum falls out
      in col 256); normalize = DVE scale of av rows by 1/(rowsum+1e-8)
    attnT via 2 PE transposes per (h, l-block)
  phase E: out = attnT.T-slices @ Wproj.T (bf16 GEMM) -> fp32
"""
import contextlib

import numpy as np
import ml_dtypes
import jax
from jax.sharding import Mesh, PartitionSpec
from jax.experimental.shard_map import shard_map

import concourse.bass as bass
import concourse.mybir as mybir
import concourse.tile as tile
from concourse.bass import AP
from concourse import bass2jax
from concourse.bass2jax import _bass_exec_p, install_neuronx_cc_hook, partition_id_tensor

F32 = mybir.dt.float32
F32R = mybir.dt.float32r
BF16 = mybir.dt.bfloat16
FP16 = mybir.dt.float16
AF = mybir.ActivationFunctionType
ALU = mybir.AluOpType


L = 1024
D = 1024
H = 4
DH = 256          # head dim
NT = 8            # 128-tiles per 1024
GW = 2047         # generator width
WW = 1152         # w_raw window width per l-block


def win_start(k):
    return max(0, min(896 - 128 * k, GW - WW))  # width WW covers needed [896-128k, 2046-128k]


def build(nc, reps=1):
    # ---------------- I/O ----------------
    xT32 = nc.dram_tensor("xT32", [NT, 128, L], F32, kind="ExternalInput")     # x.T (k,l)
    wqT = nc.dram_tensor("wqT", [NT, 128, D], F32, kind="ExternalInput")       # Wq.T (k,e)
    wvT = nc.dram_tensor("wvT", [NT, 128, D], F32, kind="ExternalInput")       # Wv.T (k,e)
    gen_d = nc.dram_tensor("gen", [H, 2, 128, GW], F32, kind="ExternalInput")  # (h, ddt, dd, j)
    wpT = nc.dram_tensor("wpT", [NT, 128, D], BF16, kind="ExternalInput")      # Wproj.T (e,f)
    out_d = nc.dram_tensor("out", [NT, 128, D], F32, kind="ExternalOutput")    # (l, f)

    with tile.TileContext(nc) as tc:
        with contextlib.ExitStack() as es:
            pconst = es.enter_context(tc.tile_pool(name="const", bufs=1))
            pout_sb = es.enter_context(tc.tile_pool(name="outer", bufs=1))
            pw2 = es.enter_context(tc.tile_pool(name="w2p", bufs=2))
            pscr = es.enter_context(tc.tile_pool(name="scr", bufs=32, space="DRAM"))
            pbig = es.enter_context(tc.tile_pool(name="big", bufs=2, space="PSUM"))
            pcum = es.enter_context(tc.tile_pool(name="cum", bufs=4, space="PSUM"))
            pavt = es.enter_context(tc.tile_pool(name="avt", bufs=2, space="PSUM"))

            # ---------------- constants ----------------
            # affine_select: out = predicate ? in_ : fill
            ident = pconst.tile([128, 128], BF16, tag="ident")
            nc.gpsimd.memset(ident[:], 1.0)
            nc.gpsimd.affine_select(   # keep where p - c == 0
                out=ident[:], in_=ident[:], compare_op=ALU.is_equal,
                fill=0.0, base=0, pattern=[[-1, 128]], channel_multiplier=1)
            # U[l', l] = 1 iff l' <= l  (keep where l - l' >= 0)
            u_tri = pconst.tile([128, 128], FP16, tag="u_tri")
            nc.gpsimd.memset(u_tri[:], 1.0)
            nc.gpsimd.affine_select(
                out=u_tri[:], in_=u_tri[:], compare_op=ALU.is_ge,
                fill=0.0, base=0, pattern=[[1, 128]], channel_multiplier=-1)

            # ---------------- whole-kernel SBUF ----------------
            # v + ones column: [jt, head, 257]
            vag = pout_sb.tile([128, NT, H, 257], BF16, tag="vag")
            for jt in range(NT):
                nc.gpsimd.memset(vag[:, jt, :, 256:257], 1.0)

            for _rep in range(reps):
                if _rep > 0:
                    tc.strict_bb_all_engine_barrier()

                w2s = []
                deferred_skew = []   # (w2, k, scr, off0) reads emitted post-barrier
                with contextlib.ExitStack() as es1:
                    p1 = es1.enter_context(tc.tile_pool(name="p1", bufs=1))
                    pgen = es1.enter_context(tc.tile_pool(name="genp", bufs=2))
                    pstg = es1.enter_context(tc.tile_pool(name="stgp", bufs=6))
                    qT = p1.tile([128, NT, L], F32R, tag="qT")    # q.T packed (e-tile, l)
                    # gen h0/h1 prefetch (before the big fp32 input loads)
                    gtiles = {}
                    for h in range(2):
                        g = pgen.tile([128, 2, GW], F32R, tag="gen", name=f"g{h}")
                        for t in range(2):
                            nc.gpsimd.dma_start(g[:, t, 0:1024], gen_d[h, t, :, 0:1024])
                            nc.gpsimd.dma_start(g[:, t, 1024:GW], gen_d[h, t, :, 1024:GW])
                        gtiles[h] = g
                    # ---------------- phase A: q GEMM ----------------
                    xt = p1.tile([128, NT, L], F32R, tag="xt")
                    wq = p1.tile([128, NT, D], F32R, tag="wqv", name="wq")
                    for k in range(NT):
                        nc.gpsimd.dma_start(xt[:, k, :], xT32[k])
                        nc.gpsimd.dma_start(wq[:, k, :], wqT[k])

                    for et in range(NT):
                        for lc in range(2):
                            ps = pbig.tile([128, 512], F32, tag="big", name="ps_q")
                            for k in range(NT):
                                nc.tensor.matmul(
                                    ps[:], wq[:, k, et * 128:(et + 1) * 128],
                                    xt[:, k, lc * 512:(lc + 1) * 512],
                                    start=(k == 0), stop=(k == NT - 1))
                            dst = qT[:, et, lc * 512:(lc + 1) * 512]
                            if (et + lc) % 2 == 0:
                                nc.vector.tensor_copy(dst, ps[:])
                            else:
                                nc.scalar.activation(dst, ps[:], AF.Copy)

                    # ---------------- phase B: window GEMMs + skew roundtrips ----------------
                    for h in range(H):
                        if h not in gtiles:
                            g = pgen.tile([128, 2, GW], F32R, tag="gen", name=f"g{h}")
                            for t in range(2):
                                nc.gpsimd.dma_start(g[:, t, 0:1024], gen_d[h, t, :, 0:1024])
                                nc.gpsimd.dma_start(g[:, t, 1024:GW], gen_d[h, t, :, 1024:GW])
                        else:
                            g = gtiles[h]
                        w2 = pw2.tile([128, NT, L], FP16, tag="w2", name=f"w2h{h}")
                        w2s.append(w2)
                        for k in range(NT):
                            s = win_start(k)
                            stg = pstg.tile([128, WW], FP16, tag="stg")
                            scr = pscr.tile([128, WW], FP16, tag="scr")
                            for ci in range(3):
                                c0 = ci * 384
                                wr = pbig.tile([128, 384], F32, tag="big", name="wr")
                                for dd in range(2):
                                    nc.tensor.matmul(
                                        wr[:],
                                        qT[:, 2 * h + dd, k * 128:(k + 1) * 128],
                                        g[:, dd, s + c0: s + c0 + 384],
                                        start=(dd == 0), stop=(dd == 1))
                                nc.scalar.activation(stg[:, c0:c0 + 384], wr[:], AF.Copy)
                                # chunked scratch write (shorter stg slot hold)
                                nc.sync.dma_start(scr[:, c0:c0 + 384], stg[:, c0:c0 + 384])
                            # skew load: w2[p, j] = scr[p, off0 - p + j]
                            # (h2/h3 w2 slots free only after h0/h1's
                            # post-barrier readers -> defer those reads)
                            off0 = 1023 - 128 * k - s
                            if h >= 2:
                                deferred_skew.append((w2, k, scr, off0))
                            else:
                                diag = AP(tensor=scr.tensor, offset=scr.offset + off0,
                                          ap=[[WW - 1, 128], [1, L]])
                                nc.sync.dma_start(w2[:, k, :], diag)

                    # ---------------- phase C: v GEMM ----------------
                    wv = p1.tile([128, NT, D], F32R, tag="wqv", name="wv")
                    for k in range(NT):
                        nc.gpsimd.dma_start(wv[:, k, :], wvT[k])
                    for lt in range(NT):
                        for ec in range(2):
                            ps = pbig.tile([128, 512], F32, tag="big", name="ps_v")
                            for k in range(NT):
                                nc.tensor.matmul(
                                    ps[:], xt[:, k, lt * 128:(lt + 1) * 128],
                                    wv[:, k, ec * 512:(ec + 1) * 512],
                                    start=(k == 0), stop=(k == NT - 1))
                            dst = vag[:, lt, 2 * ec:2 * ec + 2, 0:256]
                            src = ps.rearrange("p (h e) -> p h e", h=2)
                            nc.scalar.activation(dst, src, AF.Copy)

                # phase D/E pools reuse phase A/C SBUF space; fence so their
                # writes cannot race phase A/C readers
                tc.strict_bb_all_engine_barrier()
                for w2, k, scr, off0 in deferred_skew:
                    diag = AP(tensor=scr.tensor, offset=scr.offset + off0,
                              ap=[[WW - 1, 128], [1, L]])
                    nc.sync.dma_start(w2[:, k, :], diag)

                # ---------------- phase D: heads (cumsum^T + AV + attnT) ----------------
                with contextlib.ExitStack() as es2:
                    p2 = es2.enter_context(tc.tile_pool(name="p2", bufs=1))
                    attnT = p2.tile([128, NT, L], BF16, tag="attnT")
                    wp_w = p2.tile([128, NT, D], BF16, tag="wpw")   # Wproj.T (e-tile, f)
                    for et in range(NT):
                        nc.scalar.dma_start(wp_w[:, et, :],
                                            wpT.rearrange("e p f -> p e f")[:, et, :])

                    for h in range(H):
                        w2 = w2s[h]
                        AT = p2.tile([128, NT, L], BF16, tag="AT", bufs=2, name=f"AT{h}")
                        R = p2.tile([128, NT], F32, tag="R", bufs=2, name=f"R{h}")
                        nc.vector.memset(R[:], 0.0)

                        cum_tiles = {}   # m -> list of (psum_tile, jt0, nj)
                        ast_tiles = {}   # m -> scaled attn rows tile

                        def emit_cum(m, h=h, w2=w2, AT=AT, R=R, cum_tiles=cum_tiles):
                            tiles = []
                            for jt0 in (0, 4):
                                if jt0 > m and m == NT - 1:
                                    break  # unreachable (m=7 covers all jt)
                                pc = pcum.tile([128, 4, 128], F32, tag="cum",
                                               name=f"cum{h}_{m}_{jt0}")
                                for ji in range(4):
                                    jt = jt0 + ji
                                    if jt <= m:
                                        nc.tensor.matmul(
                                            pc[:, ji, :],
                                            w2[:, m, jt * 128:(jt + 1) * 128],
                                            u_tri[:], start=True, stop=True)
                                    elif m < NT - 1:
                                        # colsum only (feeds R for later blocks)
                                        nc.tensor.matmul(
                                            pc[:, ji, 127:128],
                                            w2[:, m, jt * 128:(jt + 1) * 128],
                                            u_tri[:, 127:128], start=True, stop=True)
                                tiles.append((pc, jt0))
                            # ACT: A'T tile = Relu(psum + R[jt]) -> bf16 SBUF
                            for pc, jt0 in tiles:
                                for ji in range(4):
                                    jt = jt0 + ji
                                    if jt <= m:
                                        nc.scalar.activation(
                                            AT[:, jt, m * 128:(m + 1) * 128],
                                            pc[:, ji, :], AF.Relu,
                                            bias=R[:, jt:jt + 1])
                            # DVE: R[jt] += block colsum (psum col 127), all jt
                            if m < NT - 1:
                                for pc, jt0 in tiles:
                                    nc.vector.tensor_tensor(
                                        R[:, jt0:jt0 + 4], R[:, jt0:jt0 + 4],
                                        pc[:, 0:4, 127], op=ALU.add)
                            # tril mask on the diagonal tile (jt == m)
                            nc.gpsimd.affine_select(
                                out=AT[:, m, m * 128:(m + 1) * 128],
                                in_=AT[:, m, m * 128:(m + 1) * 128],
                                compare_op=ALU.is_ge,
                                fill=0.0, base=0, pattern=[[1, 128]],
                                channel_multiplier=-1)
                            cum_tiles[m] = tiles

                        def emit_av(m, h=h, AT=AT, ast_tiles=ast_tiles):
                            av = pavt.tile([128, 257], F32, tag="avt", name=f"av{h}_{m}")
                            for jt in range(m + 1):
                                nc.tensor.matmul(
                                    av[:], AT[:, jt, m * 128:(m + 1) * 128],
                                    vag[:, jt, h, :],
                                    start=(jt == 0), stop=(jt == m))
                            rs = p2.tile([128, 1], F32, tag="rs", bufs=2, name=f"rs{h}_{m}")
                            nc.vector.tensor_scalar_add(rs[:], av[:, 256:257], 1e-8)
                            nc.vector.reciprocal(rs[:], rs[:])
                            ast = p2.tile([128, 256], BF16, tag="ast", bufs=2,
                                          name=f"ast{h}_{m}")
                            nc.vector.tensor_scalar_mul(ast[:], av[:, 0:256], rs[:])
                            ast_tiles[m] = ast

                        def emit_tp(m, h=h, ast_tiles=ast_tiles, attnT=attnT):
                            ast = ast_tiles.pop(m)
                            tp = pavt.tile([128, 256], BF16, tag="avt", name=f"tp{h}_{m}")
                            for e2 in range(2):
                                nc.tensor.transpose(
                                    tp[:, e2 * 128:(e2 + 1) * 128],
                                    ast[:, e2 * 128:(e2 + 1) * 128], ident[:])
                            nc.scalar.activation(
                                attnT[:, 2 * h:2 * h + 2, m * 128:(m + 1) * 128],
                                tp.rearrange("p (t l) -> p t l", t=2), AF.Copy)

                        # software-pipelined emission
                        emit_cum(0)
                        emit_cum(1)
                        emit_av(0)
                        for m in range(1, NT - 1):
                            emit_cum(m + 1)
                            emit_av(m)
                            emit_tp(m - 1)
                        emit_av(NT - 1)
                        emit_tp(NT - 2)
                        emit_tp(NT - 1)

                    # ---------------- phase E: projection ----------------
                    for lt in range(NT):
                        osb = p2.tile([128, D], F32, tag="osb", bufs=2, name=f"osb{lt}")
                        for fc in range(2):
                            ps = pbig.tile([128, 512], F32, tag="big", name="ps_o")
                            for et in range(NT):
                                nc.tensor.matmul(
                                    ps[:], attnT[:, et, lt * 128:(lt + 1) * 128],
                                    wp_w[:, et, fc * 512:(fc + 1) * 512],
                                    start=(et == 0), stop=(et == NT - 1))
                            if fc == 0:
                                nc.vector.tensor_copy(osb[:, fc * 512:(fc + 1) * 512], ps[:])
                            else:
                                nc.scalar.activation(osb[:, fc * 512:(fc + 1) * 512], ps[:], AF.Copy)
                        # split the store for lower per-queue latency
                        nc.gpsimd.dma_start(out_d[lt, :, 0:512], osb[:, 0:512])
                        nc.gpsimd.dma_start(out_d[lt, :, 512:1024], osb[:, 512:1024])

    return nc


def make_in_maps(x, generator, Wq, Wv, Wproj):
    """Full inputs -> list of 8 per-core input dicts."""
    B = x.shape[0]
    bf16 = ml_dtypes.bfloat16
    wqT = np.ascontiguousarray(Wq.T.astype(np.float32)).reshape(NT, 128, D)
    wvT = np.ascontiguousarray(Wv.T.astype(np.float32)).reshape(NT, 128, D)
    wpT = np.ascontiguousarray(Wproj.T.astype(bf16)).reshape(NT, 128, D)
    gen = np.ascontiguousarray(generator.astype(np.float32)).reshape(H, 2, 128, GW)
    maps = []
    for b in range(B):
        xT = np.ascontiguousarray(x[b].T.astype(np.float32)).reshape(NT, 128, L)
        maps.append({"xT32": xT, "wqT": wqT, "wvT": wvT, "gen": gen, "wpT": wpT})
    return maps


# ---------------------------------------------------------------------------
# post-pass: ISA instructions carry one wait slot; split extras onto NoOps


def split_excess_waits(nc, keep=1):
    n_split = 0
    for fn in nc.m.functions:
        for blk in fn.blocks:
            insts = list(blk.instructions)
            out = []
            changed = False
            for inst in insts:
                si = inst.sync_info
                waits = list(si.on_wait) if si is not None and si.on_wait else []
                if len(waits) > keep:
                    for j, w in enumerate(waits[:-keep]):
                        nop = mybir.InstNoOp(name=f"{inst.name}-ws{j}", ins=[], outs=[])
                        nop.engine = inst.engine
                        nop.sync_info = mybir.SyncInfo(on_wait=[w], on_update=[])
                        out.append(nop)
                        nc.register_instruction(nop, overwrite=True)
                    inst.sync_info = mybir.SyncInfo(
                        on_wait=waits[-keep:],
                        on_update=list(si.on_update) if si.on_update else [],
                    )
                    changed = True
                    n_split += 1
                out.append(inst)
            if changed:
                try:
                    blk.instructions = out
                except Exception:
                    blk.instructions.clear()
                    blk.instructions.extend(out)
    return n_split

# ---------------------------------------------------------------------------
class _SpmdRunner:
    def __init__(self, nc, n_cores):
        install_neuronx_cc_hook()
        self.nc = nc
        self.n_cores = n_cores
        partition_name = nc.partition_id_tensor.name if nc.partition_id_tensor else None
        in_names, out_names, out_avals = [], [], []
        for alloc in nc.m.functions[0].allocations:
            if not isinstance(alloc, mybir.MemoryLocationSet):
                continue
            name = alloc.memorylocations[0].name
            if alloc.kind == "ExternalInput":
                if name != partition_name:
                    in_names.append(name)
            elif alloc.kind == "ExternalOutput":
                out_names.append(name)
                out_avals.append(jax.core.ShapedArray(
                    tuple(alloc.tensor_shape), mybir.dt.np(alloc.dtype)))
        self.in_names, self.out_names, self.out_avals = in_names, out_names, out_avals
        n_params, n_outs = len(in_names), len(out_avals)
        all_in = in_names + out_names + ([partition_name] if partition_name else [])

        def _body(*args):
            operands = list(args)
            if partition_name is not None:
                operands.append(partition_id_tensor())
            return tuple(_bass_exec_p.bind(
                *operands, out_avals=tuple(out_avals), in_names=tuple(all_in),
                out_names=tuple(out_names), lowering_input_output_aliases=(),
                sim_require_finite=False, sim_require_nnan=False, nc=nc))

        devices = jax.devices()[:n_cores]
        assert len(devices) == n_cores, f"need {n_cores} neuron cores, have {len(jax.devices())}"
        self.mesh = Mesh(np.asarray(devices), ("core",))
        in_specs = (PartitionSpec("core"),) * (n_params + n_outs)
        out_specs = (PartitionSpec("core"),) * n_outs
        self.fn = jax.jit(shard_map(_body, mesh=self.mesh, in_specs=in_specs,
                                    out_specs=out_specs, check_rep=False),
                          keep_unused=True)
        self._dev_args = None

    def set_inputs(self, in_maps):
        n = self.n_cores
        args = [np.concatenate([np.asarray(in_maps[c][nm]) for c in range(n)], axis=0)
                for nm in self.in_names]
        for av in self.out_avals:
            args.append(np.zeros((n * av.shape[0], *av.shape[1:]), av.dtype))
        sharding = jax.sharding.NamedSharding(self.mesh, PartitionSpec("core"))
        self._dev_args = [jax.device_put(a, sharding) for a in args]

    def run(self):
        outs = self.fn(*self._dev_args)
        jax.block_until_ready(outs)
        return outs

    def outputs_np(self, outs):
        n = self.n_cores
        return [{nm: np.asarray(outs[i]).reshape(n, *self.out_avals[i].shape)[c]
                 for i, nm in enumerate(self.out_names)} for c in range(n)]


_CACHE = {}


def _get_runner(reps=1):
    if reps not in _CACHE:
        nc = bass.Bass(target_bir_lowering=False)
        build(nc, reps=reps)
        split_excess_waits(nc)
        _CACHE[reps] = _SpmdRunner(nc, 8)
    return _CACHE[reps]


def kernel(x, generator, Wq, Wv, Wproj):
    x = np.asarray(x); generator = np.asarray(generator)
    in_maps = make_in_maps(x, generator, np.asarray(Wq), np.asarray(Wv), np.asarray(Wproj))
    runner = _get_runner()
    runner.set_inputs(in_maps)
    outs = runner.outputs_np(runner.run())
    return np.stack([outs[b]["out"].reshape(L, D) for b in range(x.shape[0])]).astype(np.float32)
